# revision 2
# baseline (speedup 1.0000x reference)
"""HardTripletloss kernel for 8x Trainium2 NeuronCores (Bass, SPMD).

Strategy (feature-dim sharding, v2 — no anchor broadcast):
  - img is [49, 1048576] fp32; row 0 = anchor, rows 1:17 positives, 17:49 negatives.
  - Split the feature dim D=1048576 into 8 contiguous shards of 131072, one per core.
  - Per-core layout: D_shard = 131072 = 128 * 1024.  SBUF tile X[p, r*1024 + j]
    = shard[r, p*1024 + j]: the feature dim is split 128 ways onto partitions,
    so the anchor's chunk for partition p (X[p, 0:1024]) lives in the SAME
    partition as every other row's chunk — the per-row dot product needs no
    cross-partition data movement and no anchor broadcast.
  - HBM->SBUF DMA casts fp32->bf16 in flight (SWDGE, gpsimd-issued, 16 DMA
    engines, 4KB contiguous reads per descriptor).  7 row-tiles (7 rows each)
    overlap loading with compute.
  - Per row r: DVE scalar_tensor_tensor(X[:,r-block] * X[:,anchor-block],
    accum_out) -> per-partition dot partials [128,1] fp32.  Row 0 gives the
    anchor's squared norm.  ScalarE activation(Square, accum_out) on rows
    1..48 -> per-partition sqnorm partials.
  - Cores export dots [128,49] + sqs [128,48] fp32; the host sums partials
    (fp64) across partitions and cores and runs the tiny cos/top-k/clamp/mean
    epilogue.
"""

from contextlib import ExitStack

import numpy as np

N_ROWS = 49
D = 1048576
N_CORES = 8
D_SHARD = D // N_CORES  # 131072
P = 128                 # partitions; feature-split within a core
J = D_SHARD // P        # 1024 contiguous features per (row, partition)
R_T = 7                 # rows per load tile
N_TILES = N_ROWS // R_T  # 7

MARGIN = 0.3
K_POS = 4
K_NEG = 8
EPS = 1e-8

_CACHE: dict = {}


def _build():
    import concourse.bass as bass
    from concourse import mybir

    bf16 = mybir.dt.bfloat16
    f32 = mybir.dt.float32

    nc = bass.Bass("TRN2", target_bir_lowering=False, debug=False)
    img = nc.dram_tensor("img", [N_ROWS, D_SHARD], f32, kind="ExternalInput")
    dots = nc.dram_tensor("dots", [P, N_ROWS], f32, kind="ExternalOutput")
    sqs = nc.dram_tensor("sqs", [P, N_ROWS - 1], f32, kind="ExternalOutput")

    with ExitStack() as ctx:
        x_sb = ctx.enter_context(nc.sbuf_tensor("x_sb", [P, N_ROWS * J], bf16))
        dve_scr = ctx.enter_context(nc.sbuf_tensor("dve_scr", [P, J], bf16))
        act_scr = ctx.enter_context(nc.sbuf_tensor("act_scr", [P, J], bf16))
        dots_sb = ctx.enter_context(nc.sbuf_tensor("dots_sb", [P, N_ROWS], f32))
        sqs_sb = ctx.enter_context(nc.sbuf_tensor("sqs_sb", [P, N_ROWS - 1], f32))

        load_sem = ctx.enter_context(nc.semaphore("load_sem"))  # +16 per tile
        dve_sem = ctx.enter_context(nc.semaphore("dve_sem"))    # +1 per dot
        act_sem = ctx.enter_context(nc.semaphore("act_sem"))    # +1 per square
        out_sem = ctx.enter_context(nc.semaphore("out_sem"))    # +16 per export
        block = ctx.enter_context(nc.Block())

        # (p, r, j) -> img[r, p*J + j]; dst partition p, free offset r*J + j
        img_v = img.ap().rearrange("r (p j) -> p r j", p=P)

        @block.gpsimd
        def _(gpsimd):
            for t in range(N_TILES):
                gpsimd.dma_start(
                    out=x_sb[:, t * R_T * J : (t + 1) * R_T * J],
                    in_=img_v[:, t * R_T : (t + 1) * R_T, :],
                ).then_inc(load_sem, 16)

        @block.vector
        def _(vector):
            for r in range(N_ROWS):
                if r % R_T == 0:
                    vector.wait_ge(load_sem, 16 * (r // R_T + 1))
                nc.vector.scalar_tensor_tensor(
                    out=dve_scr[:, :],
                    in0=x_sb[:, r * J : (r + 1) * J],
                    scalar=1.0,
                    in1=x_sb[:, 0:J],
                    op0=mybir.AluOpType.mult,
                    op1=mybir.AluOpType.mult,
                    accum_out=dots_sb[:, r : r + 1],
                ).then_inc(dve_sem, 1)

        @block.scalar
        def _(scalar):
            for r in range(1, N_ROWS):
                if r % R_T == 0 or r == 1:
                    scalar.wait_ge(load_sem, 16 * (r // R_T + 1))
                nc.scalar.activation(
                    out=act_scr[:, :],
                    in_=x_sb[:, r * J : (r + 1) * J],
                    func=mybir.ActivationFunctionType.Square,
                    accum_out=sqs_sb[:, r - 1 : r],
                ).then_inc(act_sem, 1)

        @block.sync
        def _(sync):
            sync.wait_ge(dve_sem, N_ROWS)
            sync.dma_start(out=dots.ap(), in_=dots_sb[:, :]).then_inc(out_sem, 16)
            sync.wait_ge(act_sem, N_ROWS - 1)
            sync.dma_start(out=sqs.ap(), in_=sqs_sb[:, :]).then_inc(out_sem, 16)
            sync.wait_ge(out_sem, 32)

    nc.finalize()
    return nc


def _get_nc():
    if "nc" not in _CACHE:
        _CACHE["nc"] = _build()
    return _CACHE["nc"]


def _run_spmd(img: np.ndarray, **kwargs):
    """Shard the full img, run the SPMD kernel, return BassKernelResults."""
    from concourse.bass_utils import run_bass_kernel_spmd

    assert img.shape == (N_ROWS, D), img.shape
    nc = _get_nc()
    in_maps = []
    for c in range(N_CORES):
        shard = np.ascontiguousarray(
            img[:, c * D_SHARD : (c + 1) * D_SHARD], dtype=np.float32
        )
        in_maps.append({"img": shard})
    return run_bass_kernel_spmd(nc, in_maps, list(range(N_CORES)), **kwargs)


def _finish(results) -> np.ndarray:
    """Sum per-core partials and run the tiny triplet-loss epilogue on host."""
    s = np.zeros(N_ROWS, np.float64)
    q = np.zeros(N_ROWS, np.float64)
    for c in range(N_CORES):
        d = results[c]["dots"].astype(np.float64)  # [P, 49]
        sq = results[c]["sqs"].astype(np.float64)  # [P, 48]
        s += d.sum(axis=0)
        q[1:] += sq.sum(axis=0)
    q[0] = s[0]  # dot(anchor, anchor) = ||anchor||^2

    na_ = max(np.sqrt(q[0]), EPS)
    nb_ = np.maximum(np.sqrt(q[1:]), EPS)
    cos = s[1:] / (na_ * nb_)
    dist = 1.0 - cos
    d_p = dist[0:16]
    d_n = dist[16:48]
    mean_p = np.sort(d_p)[-K_POS:].mean()
    top_n = np.sort(d_n)[:K_NEG]
    loss = np.mean(np.maximum(mean_p - top_n + MARGIN, 0.0))
    return np.float32(loss)


def kernel(img: np.ndarray) -> np.ndarray:
    img = np.asarray(img)
    results = _run_spmd(img).results
    return _finish(results)


# revision 6
# speedup vs baseline: 1.0011x; 1.0011x over previous
"""HardTripletloss kernel for 8x Trainium2 NeuronCores (Bass, SPMD).

Strategy (feature-dim sharding, v3):
  - img is [49, 1048576] fp32; row 0 = anchor, rows 1:17 positives, 17:49 negatives.
  - Split the feature dim D=1048576 into 8 contiguous shards of 131072, one per core.
  - Per-core layout: D_shard = 131072 = 128 * 1024.  SBUF tile X[p, r*1024 + j]
    = shard[r, p*1024 + j]: the feature dim is split 128 ways onto partitions,
    so the anchor's chunk for partition p (X[p, 0:1024]) lives in the SAME
    partition as every other row's chunk — per-row dots need no cross-partition
    data movement and no anchor broadcast.
  - HBM->SBUF DMA casts fp32->bf16 in flight (SWDGE, gpsimd-issued, 16 DMA
    engines, 4KB contiguous reads per descriptor).  Graduated row-tiles
    (1,2,4,7,...,4,2,1) start compute ~7us earlier and cut the post-load
    compute tail to ~1.5us.
  - Per row r: DVE scalar_tensor_tensor(X[:,row r] * X[:,anchor], accum_out)
    -> per-partition dot partials [128,1] fp32 (row 0 = anchor sq norm).
    ScalarE activation(Square, accum_out) -> per-partition sqnorm partials.
    A few squares run on DVE as self-dot STTs to balance DVE (~1.23us/row)
    against ScalarE (~1.41us/row).
  - Cores export dots [128,49] + sqs [128,48] fp32; the host sums partials
    (fp64) across partitions and cores and runs the tiny cos/top-k/clamp/mean
    epilogue.
"""

from contextlib import ExitStack

import numpy as np

N_ROWS = 49
D = 1048576
N_CORES = 8
D_SHARD = D // N_CORES  # 131072
P = 128                 # partitions; feature-split within a core
J = D_SHARD // P        # 1024 contiguous features per (row, partition)

# Row-tiles: small head tiles for an early compute start, small tail tiles to
# cut the end-of-load compute tail.
TILE_ROWS = [1, 2, 4, 7, 7, 7, 7, 7, 4, 2, 1]
assert sum(TILE_ROWS) == N_ROWS

# Rows whose square runs on DVE (as a self-dot STT) instead of ScalarE, to
# balance the two engines: DVE dots cost ~1.23us/row, ScalarE squares
# ~1.41us/row, so ScalarE's 48 rows overrun the ~61us load window unless a
# few rows move over.  (GpSimd can't help: the ISA rejects TensorScalarPtr
# on the Pool engine.)
DVE_SQ = (22, 29, 36)

MARGIN = 0.3
K_POS = 4
K_NEG = 8
EPS = 1e-8

_CACHE: dict = {}


def _row_tile(r: int) -> int:
    acc = 0
    for t, n in enumerate(TILE_ROWS):
        acc += n
        if r < acc:
            return t
    raise ValueError(r)


def _build():
    import concourse.bass as bass
    from concourse import mybir

    bf16 = mybir.dt.bfloat16
    f32 = mybir.dt.float32

    nc = bass.Bass("TRN2", target_bir_lowering=False, debug=False)
    img = nc.dram_tensor("img", [N_ROWS, D_SHARD], f32, kind="ExternalInput")
    dots = nc.dram_tensor("dots", [P, N_ROWS], f32, kind="ExternalOutput")
    sqs = nc.dram_tensor("sqs", [P, N_ROWS - 1], f32, kind="ExternalOutput")

    act_rows = [r for r in range(1, N_ROWS) if r not in DVE_SQ]
    # DVE work items in row order: a dot for every row, plus self-dot squares
    # for the DVE_SQ rows.
    dve_work = []
    for r in range(N_ROWS):
        dve_work.append(("dot", r))
        if r in DVE_SQ:
            dve_work.append(("sq", r))

    with ExitStack() as ctx:
        x_sb = ctx.enter_context(nc.sbuf_tensor("x_sb", [P, N_ROWS * J], bf16))
        dve_scr = ctx.enter_context(nc.sbuf_tensor("dve_scr", [P, J], bf16))
        act_scr = ctx.enter_context(nc.sbuf_tensor("act_scr", [P, J], bf16))
        dots_sb = ctx.enter_context(nc.sbuf_tensor("dots_sb", [P, N_ROWS], f32))
        sqs_sb = ctx.enter_context(nc.sbuf_tensor("sqs_sb", [P, N_ROWS - 1], f32))

        load_sem = ctx.enter_context(nc.semaphore("load_sem"))  # +16 per tile
        dve_sem = ctx.enter_context(nc.semaphore("dve_sem"))    # +1 per DVE op
        act_sem = ctx.enter_context(nc.semaphore("act_sem"))    # +1 per square
        out_sem = ctx.enter_context(nc.semaphore("out_sem"))    # +16 per export
        block = ctx.enter_context(nc.Block())

        # (p, r, j) -> img[r, p*J + j]; dst partition p, free offset r*J + j
        img_v = img.ap().rearrange("r (p j) -> p r j", p=P)

        @block.gpsimd
        def _(gpsimd):
            row0 = 0
            for n in TILE_ROWS:
                gpsimd.dma_start(
                    out=x_sb[:, row0 * J : (row0 + n) * J],
                    in_=img_v[:, row0 : row0 + n, :],
                ).then_inc(load_sem, 16)
                row0 += n

        @block.vector
        def _(vector):
            cur = -1
            for kind, r in dve_work:
                t = _row_tile(r)
                if t > cur:
                    vector.wait_ge(load_sem, 16 * (t + 1))
                    cur = t
                other = 0 if kind == "dot" else r
                acc = dots_sb[:, r : r + 1] if kind == "dot" else sqs_sb[:, r - 1 : r]
                nc.vector.scalar_tensor_tensor(
                    out=dve_scr[:, :],
                    in0=x_sb[:, r * J : (r + 1) * J],
                    scalar=1.0,
                    in1=x_sb[:, other * J : (other + 1) * J],
                    op0=mybir.AluOpType.mult,
                    op1=mybir.AluOpType.mult,
                    accum_out=acc,
                ).then_inc(dve_sem, 1)

        @block.scalar
        def _(scalar):
            cur = -1
            for r in act_rows:
                t = _row_tile(r)
                if t > cur:
                    scalar.wait_ge(load_sem, 16 * (t + 1))
                    cur = t
                nc.scalar.activation(
                    out=act_scr[:, :],
                    in_=x_sb[:, r * J : (r + 1) * J],
                    func=mybir.ActivationFunctionType.Square,
                    accum_out=sqs_sb[:, r - 1 : r],
                ).then_inc(act_sem, 1)

        @block.sync
        def _(sync):
            sync.wait_ge(dve_sem, len(dve_work))
            sync.dma_start(out=dots.ap(), in_=dots_sb[:, :]).then_inc(out_sem, 16)
            sync.wait_ge(act_sem, len(act_rows))
            sync.dma_start(out=sqs.ap(), in_=sqs_sb[:, :]).then_inc(out_sem, 16)
            sync.wait_ge(out_sem, 32)

    nc.finalize()
    return nc


def _get_nc():
    if "nc" not in _CACHE:
        _CACHE["nc"] = _build()
    return _CACHE["nc"]


def _run_spmd(img: np.ndarray, **kwargs):
    """Shard the full img, run the SPMD kernel, return BassKernelResults."""
    from concourse.bass_utils import run_bass_kernel_spmd

    assert img.shape == (N_ROWS, D), img.shape
    nc = _get_nc()
    in_maps = []
    for c in range(N_CORES):
        shard = np.ascontiguousarray(
            img[:, c * D_SHARD : (c + 1) * D_SHARD], dtype=np.float32
        )
        in_maps.append({"img": shard})
    return run_bass_kernel_spmd(nc, in_maps, list(range(N_CORES)), **kwargs)


def _finish(results) -> np.ndarray:
    """Sum per-core partials and run the tiny triplet-loss epilogue on host."""
    s = np.zeros(N_ROWS, np.float64)
    q = np.zeros(N_ROWS, np.float64)
    for c in range(N_CORES):
        d = results[c]["dots"].astype(np.float64)  # [P, 49]
        sq = results[c]["sqs"].astype(np.float64)  # [P, 48]
        s += d.sum(axis=0)
        q[1:] += sq.sum(axis=0)
    q[0] = s[0]  # dot(anchor, anchor) = ||anchor||^2

    na_ = max(np.sqrt(q[0]), EPS)
    nb_ = np.maximum(np.sqrt(q[1:]), EPS)
    cos = s[1:] / (na_ * nb_)
    dist = 1.0 - cos
    d_p = dist[0:16]
    d_n = dist[16:48]
    mean_p = np.sort(d_p)[-K_POS:].mean()
    top_n = np.sort(d_n)[:K_NEG]
    loss = np.mean(np.maximum(mean_p - top_n + MARGIN, 0.0))
    return np.float32(loss)


def kernel(img: np.ndarray) -> np.ndarray:
    img = np.asarray(img)
    results = _run_spmd(img).results
    return _finish(results)


# revision 7
# speedup vs baseline: 1.0082x; 1.0071x over previous
"""HardTripletloss kernel for 8x Trainium2 NeuronCores (Bass, SPMD).

Strategy (2 row-groups x 4 feature-quarters, v4):
  - img is [49, 1048576] fp32; row 0 = anchor, rows 1:17 positives, 17:49 negatives.
  - 8 cores = 2 row-groups x 4 D-quarters.  Cores 0-3 take rows {0..24} (anchor
    + first 24), cores 4-7 take rows {0, 25..48}; each core gets one contiguous
    D/4 = 262144 feature quarter.  25 rows/core instead of 49 halves the
    per-row instruction count, amortizing fixed per-instruction overheads
    (DVE ~230ns, ScalarE ~340ns+280ns accum-read) over 2048-element rows.
  - Per-core layout: D_quarter = 262144 = 128 * 2048.  SBUF X[p, r*2048 + j]
    = shard[r, p*2048 + j]: the feature dim is split 128 ways onto partitions,
    so the anchor's chunk (X[p, 0:2048]) lives in the SAME partition as every
    other row's chunk — per-row dots need no cross-partition movement.
  - One SWDGE dma_start per row (fp32->bf16 cast in flight, 16 DMA engines,
    8KB contiguous reads per descriptor): compute follows the load row by row,
    so the post-load tail is a single row's compute (~2.6us).
  - DVE scalar_tensor_tensor(row * anchor, accum_out) -> dot partials [128,1]
    fp32 for rows 1..24, plus row 1's square as a self-dot (engine balance).
    ScalarE activation(Square, accum_out) -> sqnorm partials for rows 0 (the
    anchor — its squared norm) and 2..24.
  - Cores export dots [128,24] + sqs [128,25]; the host sums partials (fp64)
    across partitions and cores and runs the tiny cos/top-k/clamp/mean
    epilogue (anchor norm comes from row-group-0 cores only).
"""

from contextlib import ExitStack

import numpy as np

N_ROWS = 49
D = 1048576
N_CORES = 8
N_Q = 4                  # feature quarters
D_SHARD = D // N_Q       # 262144
P = 128                  # partitions; feature-split within a core
J = D_SHARD // P         # 2048 contiguous features per (row, partition)
R = 25                   # rows per core (anchor + 24)

MARGIN = 0.3
K_POS = 4
K_NEG = 8
EPS = 1e-8

_CACHE: dict = {}


def _build():
    import concourse.bass as bass
    from concourse import mybir

    bf16 = mybir.dt.bfloat16
    f32 = mybir.dt.float32

    nc = bass.Bass("TRN2", target_bir_lowering=False, debug=False)
    img = nc.dram_tensor("img", [R, D_SHARD], f32, kind="ExternalInput")
    dots = nc.dram_tensor("dots", [P, R - 1], f32, kind="ExternalOutput")
    sqs = nc.dram_tensor("sqs", [P, R], f32, kind="ExternalOutput")

    # DVE: dots rows 1..24 plus row 1's square (self-dot) for engine balance.
    # ACT: squares rows 0 (anchor) and 2..24.
    dve_work = [(1, "dot"), (1, "sq")] + [(r, "dot") for r in range(2, R)]
    act_rows = [0] + list(range(2, R))

    with ExitStack() as ctx:
        x_sb = ctx.enter_context(nc.sbuf_tensor("x_sb", [P, R * J], bf16))
        dve_scr = ctx.enter_context(nc.sbuf_tensor("dve_scr", [P, J], bf16))
        act_scr = ctx.enter_context(nc.sbuf_tensor("act_scr", [P, J], bf16))
        dots_sb = ctx.enter_context(nc.sbuf_tensor("dots_sb", [P, R - 1], f32))
        sqs_sb = ctx.enter_context(nc.sbuf_tensor("sqs_sb", [P, R], f32))

        load_sem = ctx.enter_context(nc.semaphore("load_sem"))  # +16 per row
        dve_sem = ctx.enter_context(nc.semaphore("dve_sem"))    # +1 per DVE op
        act_sem = ctx.enter_context(nc.semaphore("act_sem"))    # +1 per square
        out_sem = ctx.enter_context(nc.semaphore("out_sem"))    # +16 per export
        block = ctx.enter_context(nc.Block())

        # (p, r, j) -> img[r, p*J + j]; dst partition p, free offset r*J + j
        img_v = img.ap().rearrange("r (p j) -> p r j", p=P)

        @block.gpsimd
        def _(gpsimd):
            for r in range(R):
                gpsimd.dma_start(
                    out=x_sb[:, r * J : (r + 1) * J],
                    in_=img_v[:, r : r + 1, :],
                ).then_inc(load_sem, 16)

        @block.vector
        def _(vector):
            cur = -1
            for r, kind in dve_work:
                if r > cur:
                    vector.wait_ge(load_sem, 16 * (r + 1))
                    cur = r
                other = 0 if kind == "dot" else r
                acc = (
                    dots_sb[:, r - 1 : r]
                    if kind == "dot"
                    else sqs_sb[:, r : r + 1]
                )
                nc.vector.scalar_tensor_tensor(
                    out=dve_scr[:, :],
                    in0=x_sb[:, r * J : (r + 1) * J],
                    scalar=1.0,
                    in1=x_sb[:, other * J : (other + 1) * J],
                    op0=mybir.AluOpType.mult,
                    op1=mybir.AluOpType.mult,
                    accum_out=acc,
                ).then_inc(dve_sem, 1)

        @block.scalar
        def _(scalar):
            cur = -1
            for r in act_rows:
                if r > cur:
                    scalar.wait_ge(load_sem, 16 * (r + 1))
                    cur = r
                nc.scalar.activation(
                    out=act_scr[:, :],
                    in_=x_sb[:, r * J : (r + 1) * J],
                    func=mybir.ActivationFunctionType.Square,
                    accum_out=sqs_sb[:, r : r + 1],
                ).then_inc(act_sem, 1)

        @block.sync
        def _(sync):
            sync.wait_ge(dve_sem, len(dve_work))
            sync.dma_start(out=dots.ap(), in_=dots_sb[:, :]).then_inc(out_sem, 16)
            sync.wait_ge(act_sem, len(act_rows))
            sync.dma_start(out=sqs.ap(), in_=sqs_sb[:, :]).then_inc(out_sem, 16)
            sync.wait_ge(out_sem, 32)

    nc.finalize()
    return nc


def _get_nc():
    if "nc" not in _CACHE:
        _CACHE["nc"] = _build()
    return _CACHE["nc"]


_G1_ROWS = np.r_[0, 25:49]  # rows for cores 4-7: anchor + negatives tail


def _run_spmd(img: np.ndarray, **kwargs):
    """Shard the full img, run the SPMD kernel, return BassKernelResults."""
    from concourse.bass_utils import run_bass_kernel_spmd

    assert img.shape == (N_ROWS, D), img.shape
    nc = _get_nc()
    in_maps = []
    for c in range(N_CORES):
        q = c % N_Q
        rows = slice(0, R) if c < N_Q else _G1_ROWS
        shard = np.ascontiguousarray(
            img[rows, q * D_SHARD : (q + 1) * D_SHARD], dtype=np.float32
        )
        assert shard.shape == (R, D_SHARD)
        in_maps.append({"img": shard})
    return run_bass_kernel_spmd(nc, in_maps, list(range(N_CORES)), **kwargs)


def _finish(results) -> np.ndarray:
    """Sum per-core partials and run the tiny triplet-loss epilogue on host."""
    s = np.zeros(N_ROWS, np.float64)
    q = np.zeros(N_ROWS, np.float64)
    for c in range(N_CORES):
        d = results[c]["dots"].astype(np.float64).sum(axis=0)  # [24]
        sq = results[c]["sqs"].astype(np.float64).sum(axis=0)  # [25]
        if c < N_Q:
            s[1:R] += d
            q[0] += sq[0]  # anchor sq-norm: row-group-0 quarters only
            q[1:R] += sq[1:]
        else:
            s[R:] += d
            q[R:] += sq[1:]

    na_ = max(np.sqrt(q[0]), EPS)
    nb_ = np.maximum(np.sqrt(q[1:]), EPS)
    cos = s[1:] / (na_ * nb_)
    dist = 1.0 - cos
    d_p = dist[0:16]
    d_n = dist[16:48]
    mean_p = np.sort(d_p)[-K_POS:].mean()
    top_n = np.sort(d_n)[:K_NEG]
    loss = np.mean(np.maximum(mean_p - top_n + MARGIN, 0.0))
    return np.float32(loss)


def kernel(img: np.ndarray) -> np.ndarray:
    img = np.asarray(img)
    results = _run_spmd(img).results
    return _finish(results)


# revision 8
# speedup vs baseline: 1.0162x; 1.0079x over previous
"""HardTripletloss kernel for 8x Trainium2 NeuronCores (Bass, SPMD).

Strategy (2 row-groups x 4 feature-quarters, v4):
  - img is [49, 1048576] fp32; row 0 = anchor, rows 1:17 positives, 17:49 negatives.
  - 8 cores = 2 row-groups x 4 D-quarters.  Cores 0-3 take rows {0..24} (anchor
    + first 24), cores 4-7 take rows {0, 25..48}; each core gets one contiguous
    D/4 = 262144 feature quarter.  25 rows/core instead of 49 halves the
    per-row instruction count, amortizing fixed per-instruction overheads
    (DVE ~230ns, ScalarE ~340ns+280ns accum-read) over 2048-element rows.
  - Per-core layout: D_quarter = 262144 = 128 * 2048.  SBUF X[p, r*2048 + j]
    = shard[r, p*2048 + j]: the feature dim is split 128 ways onto partitions,
    so the anchor's chunk (X[p, 0:2048]) lives in the SAME partition as every
    other row's chunk — per-row dots need no cross-partition movement.
  - One SWDGE dma_start per row (fp32->bf16 cast in flight, 16 DMA engines,
    8KB contiguous reads per descriptor): compute follows the load row by row,
    so the post-load tail is a single row's compute (~2.6us).
  - DVE scalar_tensor_tensor(row * anchor, accum_out) -> dot partials [128,1]
    fp32 for rows 1..24, plus row 1's square as a self-dot (engine balance).
    ScalarE activation(Square, accum_out) -> sqnorm partials for rows 0 (the
    anchor — its squared norm) and 2..24.
  - Cores export dots [128,24] + sqs [128,25]; the host sums partials (fp64)
    across partitions and cores and runs the tiny cos/top-k/clamp/mean
    epilogue (anchor norm comes from row-group-0 cores only).
"""

from contextlib import ExitStack

import numpy as np

N_ROWS = 49
D = 1048576
N_CORES = 8
N_Q = 4                  # feature quarters
D_SHARD = D // N_Q       # 262144
P = 128                  # partitions; feature-split within a core
J = D_SHARD // P         # 2048 contiguous features per (row, partition)
R = 25                   # rows per core (anchor + 24)

MARGIN = 0.3
K_POS = 4
K_NEG = 8
EPS = 1e-8

_CACHE: dict = {}


def _build():
    import concourse.bass as bass
    from concourse import mybir

    bf16 = mybir.dt.bfloat16
    f32 = mybir.dt.float32

    nc = bass.Bass("TRN2", target_bir_lowering=False, debug=False)
    img = nc.dram_tensor("img", [R, D_SHARD], f32, kind="ExternalInput")
    dots = nc.dram_tensor("dots", [P, R - 1], f32, kind="ExternalOutput")
    sqs = nc.dram_tensor("sqs", [P, R], f32, kind="ExternalOutput")

    # DVE: dots rows 1..24 plus row 1's square (self-dot) for engine balance.
    # ACT: squares rows 0 (anchor) and 2..24.
    dve_work = [(1, "dot"), (1, "sq")] + [(r, "dot") for r in range(2, R)]
    act_rows = [0] + list(range(2, R))

    with ExitStack() as ctx:
        x_sb = ctx.enter_context(nc.sbuf_tensor("x_sb", [P, R * J], bf16))
        dve_scr = ctx.enter_context(nc.sbuf_tensor("dve_scr", [P, J], bf16))
        act_scr = ctx.enter_context(nc.sbuf_tensor("act_scr", [P, J], bf16))
        dots_sb = ctx.enter_context(nc.sbuf_tensor("dots_sb", [P, R - 1], f32))
        sqs_sb = ctx.enter_context(nc.sbuf_tensor("sqs_sb", [P, R], f32))

        load_sem = ctx.enter_context(nc.semaphore("load_sem"))  # +16 per row
        dve_sem = ctx.enter_context(nc.semaphore("dve_sem"))    # +1 per DVE op
        act_sem = ctx.enter_context(nc.semaphore("act_sem"))    # +1 per square
        out_sem = ctx.enter_context(nc.semaphore("out_sem"))    # +16 per export
        block = ctx.enter_context(nc.Block())

        # (p, r, j) -> img[r, p*J + j]; dst partition p, free offset r*J + j
        img_v = img.ap().rearrange("r (p j) -> p r j", p=P)

        @block.gpsimd
        def _(gpsimd):
            for r in range(R):
                if r >= 4:
                    # Cap outstanding load DMAs at 4 rows (~375 GB/s).  All 8
                    # cores saturating HBM lets winners run at ~420 GB/s and
                    # starves a loser core down to ~330; capping near the fair
                    # share (2.9 TB/s / 8) evens out the per-core load time.
                    gpsimd.wait_ge(load_sem, 16 * (r - 3))
                gpsimd.dma_start(
                    out=x_sb[:, r * J : (r + 1) * J],
                    in_=img_v[:, r : r + 1, :],
                ).then_inc(load_sem, 16)

        @block.vector
        def _(vector):
            cur = -1
            for r, kind in dve_work:
                if r > cur:
                    vector.wait_ge(load_sem, 16 * (r + 1))
                    cur = r
                other = 0 if kind == "dot" else r
                acc = (
                    dots_sb[:, r - 1 : r]
                    if kind == "dot"
                    else sqs_sb[:, r : r + 1]
                )
                nc.vector.scalar_tensor_tensor(
                    out=dve_scr[:, :],
                    in0=x_sb[:, r * J : (r + 1) * J],
                    scalar=1.0,
                    in1=x_sb[:, other * J : (other + 1) * J],
                    op0=mybir.AluOpType.mult,
                    op1=mybir.AluOpType.mult,
                    accum_out=acc,
                ).then_inc(dve_sem, 1)

        @block.scalar
        def _(scalar):
            cur = -1
            for r in act_rows:
                if r > cur:
                    scalar.wait_ge(load_sem, 16 * (r + 1))
                    cur = r
                nc.scalar.activation(
                    out=act_scr[:, :],
                    in_=x_sb[:, r * J : (r + 1) * J],
                    func=mybir.ActivationFunctionType.Square,
                    accum_out=sqs_sb[:, r : r + 1],
                ).then_inc(act_sem, 1)

        @block.sync
        def _(sync):
            sync.wait_ge(dve_sem, len(dve_work))
            sync.dma_start(out=dots.ap(), in_=dots_sb[:, :]).then_inc(out_sem, 16)
            sync.wait_ge(act_sem, len(act_rows))
            sync.dma_start(out=sqs.ap(), in_=sqs_sb[:, :]).then_inc(out_sem, 16)
            sync.wait_ge(out_sem, 32)

    nc.finalize()
    return nc


def _get_nc():
    if "nc" not in _CACHE:
        _CACHE["nc"] = _build()
    return _CACHE["nc"]


_G1_ROWS = np.r_[0, 25:49]  # rows for cores 4-7: anchor + negatives tail


def _run_spmd(img: np.ndarray, **kwargs):
    """Shard the full img, run the SPMD kernel, return BassKernelResults."""
    from concourse.bass_utils import run_bass_kernel_spmd

    assert img.shape == (N_ROWS, D), img.shape
    nc = _get_nc()
    in_maps = []
    for c in range(N_CORES):
        q = c % N_Q
        rows = slice(0, R) if c < N_Q else _G1_ROWS
        shard = np.ascontiguousarray(
            img[rows, q * D_SHARD : (q + 1) * D_SHARD], dtype=np.float32
        )
        assert shard.shape == (R, D_SHARD)
        in_maps.append({"img": shard})
    return run_bass_kernel_spmd(nc, in_maps, list(range(N_CORES)), **kwargs)


def _finish(results) -> np.ndarray:
    """Sum per-core partials and run the tiny triplet-loss epilogue on host."""
    s = np.zeros(N_ROWS, np.float64)
    q = np.zeros(N_ROWS, np.float64)
    for c in range(N_CORES):
        d = results[c]["dots"].astype(np.float64).sum(axis=0)  # [24]
        sq = results[c]["sqs"].astype(np.float64).sum(axis=0)  # [25]
        if c < N_Q:
            s[1:R] += d
            q[0] += sq[0]  # anchor sq-norm: row-group-0 quarters only
            q[1:R] += sq[1:]
        else:
            s[R:] += d
            q[R:] += sq[1:]

    na_ = max(np.sqrt(q[0]), EPS)
    nb_ = np.maximum(np.sqrt(q[1:]), EPS)
    cos = s[1:] / (na_ * nb_)
    dist = 1.0 - cos
    d_p = dist[0:16]
    d_n = dist[16:48]
    mean_p = np.sort(d_p)[-K_POS:].mean()
    top_n = np.sort(d_n)[:K_NEG]
    loss = np.mean(np.maximum(mean_p - top_n + MARGIN, 0.0))
    return np.float32(loss)


def kernel(img: np.ndarray) -> np.ndarray:
    img = np.asarray(img)
    results = _run_spmd(img).results
    return _finish(results)
